# revision 12
# baseline (speedup 1.0000x reference)
"""Trainium2 Bass kernel for nn_MultiHeadAttn (conv-QKV multi-head attention).

Sharding: pure data parallelism over batch B=8 -> one batch item per NeuronCore.
Per-core pipeline:
  - 3x3 SAME convs for Q, K, V computed as fp8(e4m3) DoubleRow matmuls over a
    flat-shifted padded plane.  Each operand is split into a high plane and a
    residual plane (3-term product: w_hi*x_hi + w_hi*x_lo + w_lo*x_hi) packed
    two-contraction-slices-per-matmul via DoubleRow, which the PE runs at
    0.5 cycles/row -- ~25% fewer PE cycles than the fp32r direct conv while
    keeping ~3e-3 relative error end to end.  Residual planes are pre-scaled
    by 8 with the matching 1/8 folded into the paired weight plane so all
    DoubleRow slot products share one scale.  Weights are pre-scaled by 32
    (fp8 dynamic range) and drains descale by 1/32.
  - Attention in S^T layout: S^T[tk, tq] = K Q^T per head (fp32r), mask added
    as (m-1)*1e9 pre-exp on DVE/Pool, exp on ACT with scale=1/8, PV matmul
    gives O'^T[d, tq] with softmax denominators in row 64 (from V's ones
    column); normalization via reciprocal + broadcast.
  - Output linear out = O @ Wo^T + bo (fp32r), accumulated over feature chunks.
Host-side work is layout/cast only: padding, fp8 hi/residual plane packing,
transposes of inputs.
"""

import sys

if "/opt/trn_rl_repo" not in sys.path:
    sys.path.insert(0, "/opt/trn_rl_repo")

import numpy as np

_CACHE = {}

B = 8
C = 1024          # tokens (= conv channels)
F = 1024          # features (= H*W pixels)
NH = 16           # heads
HD = 64           # head dim
L = 1158          # flat padded plane length: 34*34 + 2 (image grid at [1, 1157))

# output chunks over the padded plane interior rows (rows 1..32 of the 34x34
# padded grid, full 34-wide rows incl. pad cols): (plane_offset, n_px, n_rows,
# feat0, n_feats) where feats are the 32x32 interior pixels
CHUNKS = [(35, 476, 14, 0, 448), (511, 476, 14, 448, 448), (987, 136, 4, 896, 128)]

# tap flat-shift offsets
DELTA = [(s // 3 - 1) * 34 + (s % 3 - 1) for s in range(9)]

# contraction pair-tiles: 36 pairs of (icc, tap) covering all 72 slices once
PAIRS = []
for _icc in range(8):
    for _s0 in (0, 2, 4, 6):
        PAIRS.append(((_icc, _s0), (_icc, _s0 + 1)))
for _icc in (0, 2, 4, 6):
    PAIRS.append(((_icc, 8), (_icc + 1, 8)))
assert len(PAIRS) == 36


def _build_program(reps=1):
    from contextlib import ExitStack

    import concourse.bass as bass
    import concourse.mybir as mybir
    import concourse.tile as tile
    from concourse import bacc

    FP = mybir.dt.float32
    FR = mybir.dt.float32r
    F8 = mybir.dt.float8e4
    I32 = mybir.dt.int32

    nc = bacc.Bacc(None, target_bir_lowering=False)

    # Per-core inputs (host-prepped layouts)
    xq_d = nc.dram_tensor("xq", [C, 3 * L], F8, kind="ExternalInput")  # hi/lo8/hi8
    xk_d = nc.dram_tensor("xk", [C, 3 * L], F8, kind="ExternalInput")
    xv_d = nc.dram_tensor("xv", [C, 3 * L], F8, kind="ExternalInput")
    wq_d = nc.dram_tensor("wq", [8 * 36, 128, 768], F8, kind="ExternalInput")
    wk_d = nc.dram_tensor("wk", [8 * 36, 128, 768], F8, kind="ExternalInput")
    wv_d = nc.dram_tensor("wv", [8 * 36, 128, 768], F8, kind="ExternalInput")
    wo_d = nc.dram_tensor("wo", [F, C], mybir.dt.bfloat16, kind="ExternalInput")  # Wo^T [f, j]
    bq_d = nc.dram_tensor("bq", [C], FP, kind="ExternalInput")          # bias * 32
    bk_d = nc.dram_tensor("bk", [C], FP, kind="ExternalInput")
    bv_d = nc.dram_tensor("bv", [C], FP, kind="ExternalInput")
    bo_d = nc.dram_tensor("bo", [C], FP, kind="ExternalInput")
    mt_d = nc.dram_tensor("mt", [C, C], I32, kind="ExternalInput")      # mask^T [s, t]
    out_d = nc.dram_tensor("out", [C, C], FP, kind="ExternalOutput")

    with ExitStack() as ctx:
        tc = ctx.enter_context(tile.TileContext(nc))
        for _rep in range(reps):
            _build_body(nc, tc, bass, mybir, tile,
                        (xq_d, xk_d, xv_d, wq_d, wk_d, wv_d, wo_d,
                         bq_d, bk_d, bv_d, bo_d, mt_d, out_d))

    nc.compile()
    return nc


def _build_body(nc, tc, bass, mybir, tile, drams):
    from contextlib import ExitStack

    FP = mybir.dt.float32
    FR = mybir.dt.float32r
    F8 = mybir.dt.float8e4
    I32 = mybir.dt.int32
    AL = mybir.AluOpType
    AF = mybir.ActivationFunctionType
    PM = mybir.MatmulPerfMode
    (xq_d, xk_d, xv_d, wq_d, wk_d, wv_d, wo_d,
     bq_d, bk_d, bv_d, bo_d, mt_d, out_d) = drams

    def bcast(dram_h):
        ap = dram_h[:]
        return bass.AP(tensor=ap.tensor, offset=ap.offset, ap=[[0, 128]] + list(ap.ap))

    def ap3(base_ap, off, s1, n1, n2):
        # 3D AP [128 part, n1 (stride s1), n2 (stride 1)] at element offset off
        return bass.AP(tensor=base_ap.tensor, offset=base_ap.offset + off,
                       ap=[list(base_ap.ap[0]), [s1, n1], [1, n2]])

    with ExitStack() as ctx:
        persist = ctx.enter_context(tc.tile_pool(name="persist", bufs=1))
        BF = mybir.dt.bfloat16
        qt = persist.tile([128, 8, C], BF)        # Q^T: [f%128, f//128, t]
        kt = persist.tile([128, 8, C], BF)        # K^T
        vt = [persist.tile([128, NH, HD + 1], FR, name=f"vt{i}")
              for i in range(8)]  # V:[t%128][t//128][h, d] + ones col
        ones_g = persist.tile([128, 128], FP)
        nc.gpsimd.memset(ones_g, 1.0)
        from concourse.masks import make_identity
        ident = persist.tile([128, 128], FP)
        make_identity(nc, ident)
        ident_r = persist.tile([128, 128], FR)
        nc.vector.tensor_copy(out=ident_r, in_=ident)

        amt = persist.tile([128, 8, C], FR)  # (mask^T - 1) * 1e9

        def stage_mask_and_ones():
            for i in range(8):
                nc.vector.tensor_copy(
                    out=vt[i][:, :, HD:HD + 1],
                    in_=ones_g[:, 0:16].rearrange("p (a b) -> p a b", b=1))
            with tc.tile_pool(name="mstp", bufs=2) as mstp:
                for sc in range(8):
                    mst = mstp.tile([128, C], I32, tag="m", name="mst")
                    nc.gpsimd.dma_start(
                        out=mst, in_=mt_d[sc * 128:(sc + 1) * 128, :])
                    nc.vector.tensor_scalar(
                        out=amt[:, sc], in0=mst,
                        scalar1=1e9, scalar2=-1e9, op0=AL.mult, op1=AL.add)

        # ---------------- conv phase (fp8 DoubleRow 3-term) ----------------
        with tc.tile_pool(name="convp", bufs=1) as convp, \
                tc.tile_pool(name="stgc", bufs=2) as stgc, \
                tc.tile_pool(name="wpool", bufs=3) as wpool:
            xt = convp.tile([128, 8, 3 * L], F8)
            bqp = convp.tile([128, 8], FP)
            bkp = convp.tile([128, 8], FP)
            bvp = convp.tile([128, 8], FP)
            nc.gpsimd.dma_start(out=bqp, in_=bq_d[:].rearrange("(a p) -> p a", p=128))
            nc.gpsimd.dma_start(out=bkp, in_=bk_d[:].rearrange("(a p) -> p a", p=128))
            nc.gpsimd.dma_start(out=bvp, in_=bv_d[:].rearrange("(a p) -> p a", p=128))

            def conv8(xd, wd, drain):
                """fp8 3-term conv: psum [oc, px] per 128-oc chunk, 3 px chunks."""
                for icc in range(8):
                    nc.scalar.dma_start(
                        out=xt[:, icc], in_=xd[icc * 128:(icc + 1) * 128, :])
                xb = xt[:]
                for och in range(8):
                    ps3 = [psv.tile([128, n], FP, tag=f"v{ch}", name=f"psv{ch}")
                           for ch, (o0, n, nr, f0, nf) in enumerate(CHUNKS)]
                    wt4 = None
                    for t, ((ic0, s0), (ic1, s1)) in enumerate(PAIRS):
                        if t % 4 == 0:
                            wt4 = wpool.tile([128, 4, 768], F8, tag="w", name="wt4")
                            nc.sync.dma_start(
                                out=wt4,
                                in_=wd[och * 36 + t:och * 36 + t + 4].transpose([1, 0, 2]))
                        wb = wt4[:, t % 4]
                        lA0 = ap3(wb, 0, 128, 2, 128)      # (hi, hi8) of c0
                        lA1 = ap3(wb, 384, 128, 2, 128)    # (hi, hi8) of c1
                        lB = ap3(wb, 256, 384, 2, 128)     # (lo8 c0, lo8 c1)
                        first, last = (t == 0), (t == 35)
                        for ch, (o0, n, nr, f0, nf) in enumerate(CHUNKS):
                            # MM_A c0: w_hi*x_hi + (w_hi/8)*(8*x_lo)
                            nc.tensor.matmul(
                                ps3[ch][:, :], lA0,
                                ap3(xb, ic0 * 3 * L + o0 + DELTA[s0], L, 2, n),
                                start=first, stop=False, perf_mode=PM.DoubleRow)
                            nc.tensor.matmul(
                                ps3[ch][:, :], lA1,
                                ap3(xb, ic1 * 3 * L + o0 + DELTA[s1], L, 2, n),
                                start=False, stop=False, perf_mode=PM.DoubleRow)
                            # MM_B: (8*w_lo)*(x_hi/8) for both slices
                            sstride = (ic1 - ic0) * 3 * L + (DELTA[s1] - DELTA[s0])
                            nc.tensor.matmul(
                                ps3[ch][:, :], lB,
                                ap3(xb, ic0 * 3 * L + 2 * L + o0 + DELTA[s0],
                                    sstride, 2, n),
                                start=False, stop=last, perf_mode=PM.DoubleRow)
                    drain(och, ps3)

            def drain_v(och, ps3):
                for ch, (o0, n, nr, f0, nf) in enumerate(CHUNKS):
                    h0, nh = f0 // HD, nf // HD
                    nc.vector.tensor_scalar(
                        out=vt[och][:, h0:h0 + nh, 0:HD].rearrange(
                            "p a (b c) -> p a b c", c=32),
                        in0=ps3[ch].rearrange(
                            "p (a b c) -> p a b c", b=2, c=34)[:, :, :, 1:33],
                        scalar1=bvp[:, och:och + 1], scalar2=1.0 / 32.0,
                        op0=AL.add, op1=AL.mult)

            def make_drain_qk(bpp, dst):
                def drain(och, ps3):
                    stg_t = stgc.tile([128, C], FR, tag="st", name="stg_t")
                    for ch, (o0, n, nr, f0, nf) in enumerate(CHUNKS):
                        nc.vector.tensor_scalar(
                            out=stg_t[:, f0:f0 + nf].rearrange(
                                "p (a b) -> p a b", b=32),
                            in0=ps3[ch].rearrange(
                                "p (a b) -> p a b", b=34)[:, :, 1:33],
                            scalar1=bpp[:, och:och + 1], scalar2=1.0 / 32.0,
                            op0=AL.add, op1=AL.mult)
                    for fcc in range(8):
                        pt_ps = psT.tile([128, 128], FR, tag="t", name="pt_ps")
                        nc.tensor.transpose(
                            pt_ps, stg_t[:, fcc * 128:(fcc + 1) * 128], ident_r)
                        nc.scalar.copy(
                            out=dst[:, fcc, och * 128:(och + 1) * 128], in_=pt_ps)
                return drain

            with tc.tile_pool(name="psv", bufs=2, space="PSUM") as psv, \
                    tc.tile_pool(name="psT", bufs=2, space="PSUM") as psT:
                conv8(xq_d, wq_d, make_drain_qk(bqp, qt))
                stage_mask_and_ones()
                conv8(xk_d, wk_d, make_drain_qk(bkp, kt))
                conv8(xv_d, wv_d, drain_v)

        # ---------------- attention + output linear ----------------
        with tc.tile_pool(name="otp", bufs=1) as otp, \
                tc.tile_pool(name="linp", bufs=1) as linp:
            ot = otp.tile([128, 8, C], mybir.dt.bfloat16)  # O^T: [f%128, f//128, t]
            # prefetch output-linear weights during attention
            wos = [linp.tile([128, C], mybir.dt.bfloat16, name=f"wos{i}") for i in range(8)]
            for fc in range(8):
                nc.sync.dma_start(
                    out=wos[fc], in_=wo_d[fc * 128:(fc + 1) * 128, :])
            bob = linp.tile([128, C], FP)
            nc.gpsimd.dma_start(out=bob, in_=bcast(bo_d))

            with tc.tile_pool(name="attp", bufs=1) as attp, \
                    tc.tile_pool(name="ptp", bufs=4) as ptp, \
                    tc.tile_pool(name="smallp", bufs=2) as smallp, \
                    tc.tile_pool(name="dscp", bufs=4, space="DRAM") as dscp, \
                    tc.tile_pool(name="psS", bufs=4, space="PSUM") as psS, \
                    tc.tile_pool(name="psO", bufs=2, space="PSUM") as psO:
                for fc in range(8):
                    po = {}
                    for hh, pb in ((2 * fc, 0), (2 * fc + 1, 64)):
                        po[hh] = psO.tile([65, C], FP, tag="o", name=f"po{hh}")
                    for tkc in range(8):
                        for hh, pb in ((2 * fc, 0), (2 * fc + 1, 64)):
                            ptt = ptp.tile([128, C], FR, tag="pt", name="ptt")
                            for qh in range(2):
                                s_ps = psS.tile([128, 512], FP, tag="s", name="sps")
                                nc.tensor.matmul(
                                    s_ps,
                                    kt[pb:pb + 64, fc, tkc * 128:(tkc + 1) * 128],
                                    qt[pb:pb + 64, fc, qh * 512:(qh + 1) * 512],
                                    start=True, stop=(qh == 1))
                                if qh == 0:
                                    nc.tensor.matmul(
                                        s_ps, ident_r,
                                        amt[:, tkc, qh * 512:(qh + 1) * 512],
                                        start=False, stop=True)
                                    nc.scalar.activation(
                                        out=ptt[:, 0:512], in_=s_ps,
                                        func=AF.Exp, scale=0.125)
                                else:
                                    nc.vector.tensor_add(
                                        ptt[:, 512:C], s_ps,
                                        amt[:, tkc, qh * 512:(qh + 1) * 512])
                                    nc.scalar.activation(
                                        out=ptt[:, 512:C], in_=ptt[:, 512:C],
                                        func=AF.Exp, scale=0.125)
                            for qh in range(2):
                                nc.tensor.matmul(
                                    po[hh][:, qh * 512:(qh + 1) * 512],
                                    vt[tkc][:, hh].bitcast(FR),
                                    ptt[:, qh * 512:(qh + 1) * 512].bitcast(FR),
                                    start=(tkc == 0), stop=(tkc == 7))
                    for hh, pb in ((2 * fc, 0), (2 * fc + 1, 64)):
                        r1 = smallp.tile([1, C], FP, tag="r1", name="r1")
                        nc.vector.reciprocal(out=r1, in_=po[hh][64:65, :])
                        dsc = dscp.tile([1, C], FP, tag="d", name="dsc")
                        nc.gpsimd.dma_start(out=dsc, in_=r1)
                        rbs = smallp.tile([64, C], FP, tag="rbs", name="rbs")
                        dap = dsc[0:1, :]
                        nc.gpsimd.dma_start(out=rbs, in_=bass.AP(
                            tensor=dap.tensor, offset=dap.offset,
                            ap=[[0, 64]] + list(dap.ap)[1:]))
                        if pb == 0:
                            nc.vector.tensor_mul(
                                ot[0:64, fc, :], po[hh][0:64, :], rbs)
                        else:
                            stage = smallp.tile([64, C], mybir.dt.bfloat16, tag="sg", name="sg")
                            nc.vector.tensor_mul(
                                stage, po[hh][0:64, :], rbs)
                            nc.gpsimd.dma_start(
                                out=ot[64:128, fc, :], in_=stage)

            with tc.tile_pool(name="stg", bufs=2) as stg, \
                    tc.tile_pool(name="psL", bufs=3, space="PSUM") as psL:
                for tcc in range(8):
                    pls = psL.tile([128, C], FP, tag="l", name="psl")
                    for fc in range(8):
                        lhsT = ot[:, fc, tcc * 128:(tcc + 1) * 128]
                        for jh in range(2):
                            nc.tensor.matmul(
                                pls[:, jh * 512:(jh + 1) * 512],
                                lhsT,
                                wos[fc][:, jh * 512:(jh + 1) * 512],
                                start=(fc == 0), stop=(fc == 7))
                    so = stg.tile([128, C], FP, tag="so", name="so")
                    nc.vector.tensor_add(so, pls, bob)
                    nc.sync.dma_start(
                        out=out_d[tcc * 128:(tcc + 1) * 128, :], in_=so)


def _q8(x):
    import ml_dtypes
    return np.clip(x, -240, 240).astype(ml_dtypes.float8_e4m3)


def _prep_x8(x):
    # [C, 32, 32] f32 -> [C, 3*L] fp8 planes (hi, lo8, hi8)
    xp = np.zeros((C, 34, 34), np.float32)
    xp[:, 1:33, 1:33] = x
    flat = np.zeros((C, L), np.float32)
    flat[:, 1:1157] = xp.reshape(C, 1156)
    hi = _q8(flat)
    hif = hi.astype(np.float32)
    lo8 = _q8((flat - hif) * 8.0)
    hi8 = _q8(hif / 8.0)
    return np.stack([hi, lo8, hi8], axis=1).reshape(C, 3 * L)


def _prep_w8(W):
    # [O, I, 3, 3] -> [8*36, 128, 768] fp8: per (och, pair): [i][c2][hi,hi8,lo8][oc]
    Ws = np.asarray(W, np.float32) * 32.0
    hi = _q8(Ws)
    hif = hi.astype(np.float32)
    hi8 = _q8(hif / 8.0)
    lo8 = _q8((Ws - hif) * 8.0)
    P = np.stack([hi, hi8, lo8], axis=0)          # [3, O, I, 3, 3]
    P = P.reshape(3, 8, 128, 8, 128, 9)           # [3, och, o, icc, i, s]
    out = np.empty((8, 36, 128, 2, 3, 128), P.dtype)
    for t, pair in enumerate(PAIRS):
        for j, (icc, s) in enumerate(pair):
            # [3, och, o, i] -> [och, i, 3, o]
            out[:, t, :, j, :, :] = P[:, :, :, icc, :, s].transpose(1, 3, 0, 2)
    return np.ascontiguousarray(out.reshape(8 * 36, 128, 768))


def get_program(reps=1):
    key = ("nc", reps)
    if key not in _CACHE:
        _CACHE[key] = _build_program(reps)
    return _CACHE[key]


def make_in_maps(q, k, v, Wq, bq, Wk, bk, Wv, bv, Wo, bo, mask):
    wq = _prep_w8(np.asarray(Wq))
    wk = _prep_w8(np.asarray(Wk))
    wv = _prep_w8(np.asarray(Wv))
    import ml_dtypes
    wo = np.ascontiguousarray(np.asarray(Wo).T.astype(ml_dtypes.bfloat16))
    bq, bk, bv = (np.ascontiguousarray(np.asarray(b), dtype=np.float32) * 32.0
                  for b in (bq, bk, bv))
    bo = np.ascontiguousarray(np.asarray(bo), dtype=np.float32)
    in_maps = []
    for b in range(B):
        in_maps.append({
            "xq": _prep_x8(np.asarray(q[b]).reshape(C, 32, 32)),
            "xk": _prep_x8(np.asarray(k[b]).reshape(C, 32, 32)),
            "xv": _prep_x8(np.asarray(v[b]).reshape(C, 32, 32)),
            "wq": wq, "wk": wk, "wv": wv, "wo": wo,
            "bq": bq, "bk": bk, "bv": bv, "bo": bo,
            "mt": np.ascontiguousarray(np.asarray(mask[b]).T),
        })
    return in_maps


def run(inputs, trace=False, **kw):
    from concourse.bass_utils import run_bass_kernel_spmd

    nc = get_program()
    in_maps = make_in_maps(**inputs)
    res = run_bass_kernel_spmd(nc, in_maps, list(range(B)), trace=trace, **kw)
    out = np.stack([res.results[i]["out"] for i in range(B)], axis=0)
    return out, res


def kernel(**inputs) -> np.ndarray:
    out, _ = run(inputs, trace=False)
    return out
